# revision 28
# baseline (speedup 1.0000x reference)
"""Trainium2 Bass kernel for the CPA block (sparse/efficient attention).

Strategy
--------
Data parallel over batch: B=128 -> 16 batch elements per NeuronCore, all
parameters replicated (folded on host into a handful of small matrices).
Inputs are converted to bf16 on the host (rel-err budget 2e-2 leaves room).

Pass 1 runs a 4-stage software pipeline (front / back1a / back1b / back2,
skewed by one batch each) so no engine ever blocks on a result produced
earlier in the same batch:
  front(b):  load f2/f3 tokens-first into one tile; per-tile bn_stats +
             hand-rolled even/odd aggregation (GpSimd smalls); rstd via
             fast inverse-sqrt (bit trick + 1 Newton step) fully on DVE;
             f4 handled in 2-batch pairs (64 tokens each).
  back1a(b): normalize (tiles split DVE tensor_scalar / ACT Identity with
             scale+bias); z -> zT channels-first via ONE DMA XBAR
             transpose (no PE transposes, no PSUM copies); k3/v3 mms with
             biases via identity matmuls (emitted before q so ACT's
             exp/scale latency hides behind the q matmuls); exp(k);
             per-head sums; 1/s folded into v via ACT Copy+scale;
             q1/q2 = Wq.T @ zT + ident-mm bias; exp with accum_out
             giving softmax denominators for free.
  back1b(b): Gram matmuls; masked block-diag; fold with rp_w on PE; 1/S
             via ACT Copy+scale -> M32/M42; attention TOKENS-FIRST (per
             128-token tile ap += eqT@M32 + eqT@M42 + rank-1 rp_b);
             x_bf = f2 + ap; LN4 stats.
  back2(b):  zx normalize; zxT via DMA transpose.
Pass 2 (Gelu act table) runs a 3-stage pipeline:
  p2a: ff1 -> Gelu ([128,2,512] 2-bank chunks, bias via ACT port)
  p2b: ff2 channels-first (8 wide mms + single-group rank-1 ff2_b),
       bf16 copy, DMA-transpose back to tokens-first
  p2c: out = x_bf + hT -> DMA.

All matmuls bf16 (fp32 PSUM accumulate); residual stream bf16 (rel-err
budget 2e-2; measured ~1.8e-3).
"""

import os

import ml_dtypes
import numpy as np

NB = 16  # batch elements per core
NCORES = 8
EPS = 1e-5
N2, N3, N4, D, MLP = 1024, 256, 64, 128, 512
T2, T3 = N2 // 128, N3 // 128

_PROGRAM = None
LAST_RESULTS = None


def _build_program(nb=NB):
    from contextlib import ExitStack

    import concourse.bacc as bacc
    import concourse.mybir as mybir
    import concourse.tile as tile

    f32 = mybir.dt.float32
    bf16 = mybir.dt.bfloat16
    i32 = mybir.dt.int32
    A = mybir.ActivationFunctionType
    Alu = mybir.AluOpType
    X = mybir.AxisListType.X

    class _Bacc(bacc.Bacc):
        _ACT_SETS = {"natural_log_exp_and_others", "gelu_and_others"}

        def insert_act_table_loads(self):
            import bass_rust as _bass_rust

            from concourse.hw_specs import get_activation_tables

            has_activation = any(
                isinstance(i, mybir.InstActivation)
                for b in self.main_func.blocks
                for i in b.instructions
            )
            if not has_activation:
                return
            tables = [
                (name, (fns if name in self._ACT_SETS else set()))
                for name, fns in get_activation_tables(self.m.arch).items()
            ]
            _bass_rust.insert_act_table_loads(self, tables)

    nc = _Bacc("TRN2", target_bir_lowering=False, debug=False)

    def din(name, shape, dt=bf16):
        return nc.dram_tensor(name, shape, dt, kind="ExternalInput").ap()

    f2d = din("f2", [nb, N2, D])
    f3d = din("f3", [nb, N3, D])
    f4d = din("f4", [nb, N4, D])
    wq1d = din("wq1", [D, D])
    wq2d = din("wq2", [D, D])
    wkv1d = din("wkv1", [D, 2 * D])
    wkv2d = din("wkv2", [D, 2 * D])
    wrpd = din("wrp", [2, D, D])
    wff1d = din("wff1", [D, MLP])
    wff2d = din("wff2", [4, D, D])
    bq1td = din("bq1t", [D, N2])
    bq2td = din("bq2t", [D, N2])
    bkv3d = din("bkv3", [T3, 128, 2 * D])
    bkv4d = din("bkv4", [128, 2 * D])
    rpbrd = din("rpbr", [1, 4, D])
    ff2brd = din("ff2br", [1, 4, D])
    onesrd = din("onesr", [1, 128])
    identd = din("ident", [128, 128])
    ff1bd = din("ff1b", [D, 4], f32)
    outd = nc.dram_tensor("out", [nb, N2, D], f32, kind="ExternalOutput").ap()

    LNEPS = 128.0 * EPS
    C128 = 0.5 * float(np.log(128.0))

    with tile.TileContext(nc) as tc, ExitStack() as ctx:
        consts = ctx.enter_context(tc.tile_pool(name="consts", bufs=1))
        state = ctx.enter_context(tc.tile_pool(name="state", bufs=1))
        work = ctx.enter_context(tc.tile_pool(name="work", bufs=2))
        zpool = ctx.enter_context(tc.tile_pool(name="zpool", bufs=4))
        ftpool = ctx.enter_context(tc.tile_pool(name="ftpool", bufs=3))
        pwork = ctx.enter_context(tc.tile_pool(name="pwork", bufs=2))
        small = ctx.enter_context(tc.tile_pool(name="small", bufs=2))

        def cload(name, shape, dt, src):
            t = consts.tile(shape, dt, name=name)
            nc.sync.dma_start(t, src)
            return t

        wq1 = cload("wq1_sb", [D, D], bf16, wq1d)
        wq2 = cload("wq2_sb", [D, D], bf16, wq2d)
        wkv1 = cload("wkv1_sb", [D, 2 * D], bf16, wkv1d)
        wkv2 = cload("wkv2_sb", [D, 2 * D], bf16, wkv2d)
        wrp = consts.tile([D, 2, D], bf16, name="wrp_sb")
        nc.sync.dma_start(wrp, wrpd.rearrange("g d e -> d g e"))
        wff1 = cload("wff1_sb", [D, MLP], bf16, wff1d)
        wff2 = consts.tile([128, 4, 128], bf16, name="wff2_sb")
        nc.sync.dma_start(wff2, wff2d.rearrange("j k m -> k j m"))
        bq1t = cload("bq1t_sb", [D, N2], bf16, bq1td)
        bq2t = cload("bq2t_sb", [D, N2], bf16, bq2td)
        bkv3 = consts.tile([128, T3, 2 * D], bf16, name="bkv3_sb")
        nc.sync.dma_start(bkv3, bkv3d.rearrange("t p d -> p t d"))
        bkv4 = cload("bkv4_sb", [128, 2 * D], bf16, bkv4d)
        rpbr4 = cload("rpbr_sb", [1, 4, D], bf16, rpbrd)
        ff2br4 = cload("ff2br_sb", [1, 4, D], bf16, ff2brd)
        onesr = cload("onesr_sb", [1, 128], bf16, onesrd)
        ident = cload("ident_sb", [128, 128], bf16, identd)
        ff1b = cload("ff1b_sb", [D, 4], f32, ff1bd)

        magic = consts.tile([128, 16], i32, name="magic")
        nc.vector.memset(magic, 0x5F3759DF)

        x_bf = state.tile([128, nb, T2, 128], bf16, name="x_bf")
        zxT_all = state.tile([128, nb, T2, 128], bf16, name="zxT_all")

        def ln_stats(src_view, G, tag, eng_diy=None):
            """src [128, G, 128] -> (m [128,G] f32, rstd [128,G] f32).

            bn_stats in <=4-tile chunks, even/odd aggregation by hand,
            rstd on ACT with folded constants.
            """
            eng = eng_diy or nc.vector
            st = small.tile([128, G, 6], f32, tag=f"st{tag}", name=f"st{tag}")
            for i in range(G):
                nc.vector.bn_stats(st[:, i, :], src_view[:, i, :])
            sm = small.tile([128, 4, G], f32, tag=f"ag{tag}", name=f"ag{tag}")
            # sm[:,0]=m_e+m_o  sm[:,1]=m_e-m_o  sm[:,2]=32*d^2  sm[:,3]=v128
            eng.tensor_tensor(sm[:, 0, :], st[:, :, 1], st[:, :, 4], Alu.add)
            eng.tensor_tensor(sm[:, 1, :], st[:, :, 1], st[:, :, 4],
                              Alu.subtract)
            eng.tensor_tensor(sm[:, 2, :], sm[:, 1, :], sm[:, 1, :], Alu.mult)
            eng.tensor_scalar(sm[:, 2, :], sm[:, 2, :], 32.0, None, Alu.mult)
            eng.tensor_tensor(sm[:, 3, :], st[:, :, 2], st[:, :, 5], Alu.add)
            v128 = small.tile([128, G], f32, tag=f"v{tag}", name=f"v{tag}")
            eng.tensor_tensor(v128, sm[:, 3, :], sm[:, 2, :], Alu.add)
            m = small.tile([128, G], f32, tag=f"m{tag}", name=f"m{tag}")
            eng.tensor_scalar(m, sm[:, 0, :], 0.5, None, Alu.mult)
            # rstd = 1/sqrt(var + eps) via bit trick + one Newton step, all
            # on DVE (avoids a DVE->ACT->DVE latency round trip per batch).
            wt = small.tile([128, G], f32, tag=f"w{tag}", name=f"w{tag}")
            nc.vector.tensor_scalar(wt, v128, LNEPS, 1.0 / 128.0,
                                    Alu.add, Alu.mult)
            it = small.tile([128, G], i32, tag=f"i{tag}", name=f"i{tag}")
            nc.vector.tensor_scalar(it, wt.bitcast(i32), 1, None,
                                    Alu.logical_shift_right)
            y = small.tile([128, G], f32, tag=f"y{tag}", name=f"y{tag}")
            nc.vector.tensor_tensor(y.bitcast(i32), magic[:, :G], it,
                                    Alu.subtract)
            y2 = small.tile([128, G], f32, tag=f"y2{tag}", name=f"y2{tag}")
            nc.vector.tensor_tensor(y2, y, y, Alu.mult)
            h = small.tile([128, G], f32, tag=f"h{tag}", name=f"h{tag}")
            nc.vector.scalar_tensor_tensor(h, wt, -0.5, y2, Alu.mult, Alu.mult)
            rstd = small.tile([128, G], f32, tag=f"r{tag}", name=f"r{tag}")
            nc.vector.scalar_tensor_tensor(rstd, h, 1.5, y, Alu.add, Alu.mult)
            negmr = small.tile([128, G], f32, tag=f"n{tag}", name=f"n{tag}")
            nc.vector.scalar_tensor_tensor(negmr, m, -1.0, rstd,
                                           Alu.mult, Alu.mult)
            return m, rstd, negmr

        # ---------------- pass 1 (3-stage software pipeline) ----------------
        fronts, mids, pairs, backs = {}, {}, {}, {}
        with tc.tile_pool(name="psq", bufs=2, space="PSUM") as psq, \
             tc.tile_pool(name="pssm", bufs=2, space="PSUM") as pssm, \
             tc.tile_pool(name="psx", bufs=2, space="PSUM") as psx:

            def norm_split(dst, src_v, G, m, r, negmr):
                for t in range(G):
                    if t % 2 == 0:
                        nc.vector.tensor_scalar(dst[:, t, :], src_v[:, t, :],
                                                m[:, t : t + 1],
                                                r[:, t : t + 1],
                                                Alu.subtract, Alu.mult)
                    else:
                        nc.scalar.activation(dst[:, t, :], src_v[:, t, :],
                                             A.Identity,
                                             scale=r[:, t : t + 1],
                                             bias=negmr[:, t : t + 1])

            def front(b):
                """Load + LN stats (results consumed one stage later)."""
                ft = ftpool.tile([128, T2 + T3, 128], bf16, tag="ft", name="ft")
                nc.sync.dma_start(ft[:, :T2, :],
                                  f2d[b].rearrange("(t p) d -> p t d", p=128))
                nc.sync.dma_start(ft[:, T2:, :],
                                  f3d[b].rearrange("(t p) d -> p t d", p=128))
                m23, r23, n23 = ln_stats(ft, T2 + T3, "23", eng_diy=nc.gpsimd)
                fronts[b] = (ft, m23, r23, n23)
                if b % 2 == 0:
                    f4p = pwork.tile([128, 1, 128], bf16, tag="f4p", name="f4p")
                    nc.sync.dma_start(f4p[:, 0, :],
                                      f4d[b : b + 2].rearrange("b n d -> (b n) d"))
                    m4, r4, _n4 = ln_stats(f4p, 1, "4", eng_diy=nc.gpsimd)
                    pairs[b // 2] = [f4p, m4, r4]

            def back1a(b):
                """Normalize + transpose + kv3/q matmuls + exps."""
                ft, m23, r23, n23 = fronts.pop(b)
                if b % 2 == 0:
                    # f4 pair: normalize + kv4 (stats ready since front(b))
                    f4p, m4, r4 = pairs[b // 2]
                    z4p = zpool.tile([128, 128], bf16, tag="z4p", name="z4p")
                    nc.vector.tensor_scalar(z4p, f4p[:, 0, :], m4[:, 0:1],
                                            r4[:, 0:1], Alu.subtract, Alu.mult)
                    z4T = pwork.tile([128, 128], bf16, tag="z4T", name="z4T")
                    nc.sync.dma_start_transpose(z4T, z4p)
                    kv4p = pssm.tile([128, 2 * D], f32, tag="sm", name="kv4p")
                    nc.tensor.matmul(kv4p, z4T, wkv2, start=True, stop=False)
                    nc.tensor.matmul(kv4p, ident, bkv4, start=False, stop=True)
                    ek4 = pwork.tile([128, 128], bf16, tag="ek4", name="ek4")
                    nc.scalar.activation(ek4, kv4p[:, 0:D], A.Exp)
                    s4 = pwork.tile([128, 1, 2], f32, tag="s4", name="s4")
                    nc.vector.tensor_reduce(
                        s4, ek4.rearrange("p (o h e) -> p o h e", o=1, h=2),
                        axis=X, op=Alu.add)
                    nc.vector.reciprocal(s4, s4)
                    v4s = pwork.tile([128, 128], bf16, tag="v4s", name="v4s")
                    for h in range(2):
                        nc.scalar.activation(
                            v4s[:, h * 64 : (h + 1) * 64],
                            kv4p[:, D + h * 64 : D + (h + 1) * 64],
                            A.Copy, scale=s4[:, 0, h : h + 1])
                    pairs[b // 2] = {"ek4": ek4, "v4s": v4s}

                z = zpool.tile([128, T2 + T3, 128], bf16, tag="z", name="z")
                norm_split(z, ft, T2 + T3, m23, r23, n23)
                zT = work.tile([128, T2 + T3, 128], bf16, tag="zT", name="zT")
                nc.sync.dma_start_transpose(
                    zT, z.rearrange("p t d -> p (t d)"))

                # k3/v3 first (so ACT's exp/scale latency hides behind q-mms)
                kv3p = pssm.tile([128, T3, 2 * D], f32, tag="sm", name="kv3p")
                for t in range(T3):
                    nc.tensor.matmul(kv3p[:, t, :], zT[:, T2 + t, :], wkv1,
                                     start=True, stop=False)
                    nc.tensor.matmul(kv3p[:, t, :], ident, bkv3[:, t, :],
                                     start=False, stop=True)
                ek3 = work.tile([128, T3, 128], bf16, tag="ek3", name="ek3")
                nc.scalar.activation(ek3, kv3p[:, :, 0:D], A.Exp)
                s3 = small.tile([128, T3, 2], f32, tag="s3", name="s3")
                nc.vector.tensor_reduce(
                    s3, ek3.rearrange("p t (h e) -> p t h e", h=2),
                    axis=X, op=Alu.add)
                nc.vector.reciprocal(s3, s3)
                v3s = work.tile([128, T3, 128], bf16, tag="v3s", name="v3s")
                for t in range(T3):
                    for h in range(2):
                        nc.scalar.activation(
                            v3s[:, t, h * 64 : (h + 1) * 64],
                            kv3p[:, t, D + h * 64 : D + (h + 1) * 64],
                            A.Copy, scale=s3[:, t, h : h + 1])

                # q projections + exp with accumulated denominators
                S = small.tile([128, 2], f32, tag="S", name="S")
                eqs = []
                for qi, (wq, bqt) in enumerate(((wq1, bq1t), (wq2, bq2t))):
                    qp = psq.tile([128, 2, 512], f32, tag="q", name="qp")
                    for c in range(2):
                        nc.tensor.matmul(qp[:, c, :], wq,
                                         zT[:, c * 4 : (c + 1) * 4, :],
                                         start=True, stop=False)
                        nc.tensor.matmul(qp[:, c, :], ident,
                                         bqt[:, c * 512 : (c + 1) * 512],
                                         start=False, stop=True)
                    eq = work.tile([128, 2, 512], bf16, tag=f"eq{qi}",
                                   name=f"eq{qi}")
                    nc.scalar.activation(eq, qp, A.Exp,
                                         accum_out=S[:, qi : qi + 1])
                    eqs.append(eq)
                rS = small.tile([128, 2], f32, tag="rS", name="rS")
                nc.vector.reciprocal(rS, S)
                mids[b] = (ft, eqs[0], eqs[1], ek3, v3s, rS)

            def back1b(b):
                ft, eq1, eq2, ek3, v3s, rS = mids.pop(b)
                pair = pairs[b // 2]
                # Gram matrices (transposed): g[e,d] = sum_tok v[tok,e] k[tok,d]
                gp = pssm.tile([128, 2, 128], f32, tag="sm", name="gp")
                for t in range(T3):
                    nc.tensor.matmul(gp[:, 0, :], v3s[:, t, :], ek3[:, t, :],
                                     start=(t == 0), stop=(t == T3 - 1))
                off = (b % 2) * 64
                nc.tensor.matmul(gp[:, 1, :],
                                 pair["v4s"][off : off + 64, :],
                                 pair["ek4"][off : off + 64, :],
                                 start=True, stop=True)
                # mask to block-diagonal (GpSimd), fold rp_w and 1/S on PE/DVE
                gmb = work.tile([128, 2, 128], bf16, tag="gmb", name="gmb")
                nc.gpsimd.memset(gmb, 0)
                for h in range(2):
                    nc.scalar.activation(
                        gmb[h * 64 : (h + 1) * 64, :, h * 64 : (h + 1) * 64],
                        gp[h * 64 : (h + 1) * 64, :, h * 64 : (h + 1) * 64],
                        A.Copy)
                mp = pssm.tile([128, 2, 128], f32, tag="sm", name="mp")
                for gi in range(2):
                    nc.tensor.matmul(mp[:, gi, :], gmb[:, gi, :], wrp[:, gi, :],
                                     start=True, stop=True)
                ms = work.tile([128, 2, 128], bf16, tag="ms", name="ms")
                for gi in range(2):
                    nc.scalar.activation(ms[:, gi, :], mp[:, gi, :],
                                         A.Copy, scale=rS[:, gi : gi + 1])

                # attn tokens-first + residual; x_bf = f2 + attn + rp_b
                for half in range(2):
                    ap = psx.tile([128, 4, 128], f32, tag="x", name="ap")
                    for q in range(4):
                        t = half * 4 + q
                        c, w = t // 4, (t % 4) * 128
                        nc.tensor.matmul(ap[:, q, :],
                                         eq1[:, c, w : w + 128], ms[:, 0, :],
                                         start=True, stop=False)
                        nc.tensor.matmul(ap[:, q, :],
                                         eq2[:, c, w : w + 128], ms[:, 1, :],
                                         start=False, stop=False)
                        nc.tensor.matmul(ap[:, q, :], onesr, rpbr4[:, 0, :],
                                         start=False, stop=True)
                    nc.vector.tensor_tensor(
                        x_bf[:, b, half * 4 : (half + 1) * 4, :]
                            .rearrange("p t d -> p (t d)"),
                        ft[:, half * 4 : (half + 1) * 4, :]
                            .rearrange("p t d -> p (t d)"),
                        ap.rearrange("p t d -> p (t d)"), Alu.add)

                # LN4 stats on x_bf (normalize happens one stage later)
                mx, rx, nx = ln_stats(x_bf[:, b], T2, "x", eng_diy=nc.gpsimd)
                backs[b] = (mx, rx, nx)

            def back2(b):
                mx, rx, nx = backs.pop(b)
                zx = zpool.tile([128, T2, 128], bf16, tag="zx", name="zx")
                norm_split(zx, x_bf[:, b], T2, mx, rx, nx)
                nc.sync.dma_start_transpose(
                    zxT_all[:, b], zx.rearrange("p t d -> p (t d)"))

            for i in range(nb + 3):
                if i < nb:
                    front(i)
                if 3 <= i <= nb + 2:
                    back2(i - 3)
                if 2 <= i <= nb + 1:
                    back1b(i - 2)
                if 1 <= i <= nb:
                    back1a(i - 1)

        # ---------------- pass 2 (3-stage pipeline) ----------------
        gjss, hts = {}, {}
        with tc.tile_pool(name="psg", bufs=2, space="PSUM") as psg, \
             tc.tile_pool(name="psh", bufs=2, space="PSUM") as psh:

            def p2a(b):
                zxT = zxT_all[:, b]
                gjs = []
                for j in range(4):
                    gpp = psg.tile([128, 2, 512], f32, tag="g", name="gpp")
                    for c in range(2):
                        nc.tensor.matmul(gpp[:, c, :],
                                         wff1[:, j * 128 : (j + 1) * 128],
                                         zxT[:, c * 4 : (c + 1) * 4, :],
                                         start=True, stop=True)
                    gj = work.tile([128, 2, 512], bf16, tag=f"gj{j}",
                                   name=f"gj{j}")
                    nc.scalar.activation(gj, gpp, A.Gelu,
                                         bias=ff1b[:, j : j + 1])
                    gjs.append(gj)
                gjss[b] = gjs

            def p2b(b):
                # h channels-first: 8 wide matmuls, then bf16 copy + DMA
                # transpose back to tokens-first.
                gjs = gjss.pop(b)
                hq = psh.tile([128, 2, 512], f32, tag="h", name="hq")
                for c in range(2):
                    for j in range(4):
                        nc.tensor.matmul(hq[:, c, :], wff2[:, j, :],
                                         gjs[j][:, c, :],
                                         start=(j == 0), stop=False)
                    nc.tensor.matmul(
                        hq[:, c, :], onesr,
                        ff2br4.rearrange("o f d -> o (f d)"),
                        start=False, stop=True)
                hcf = work.tile([128, 2, 512], bf16, tag="hcf", name="hcf")
                nc.vector.tensor_copy(hcf, hq)
                ht = zpool.tile([128, T2, 128], bf16, tag="ht", name="ht")
                nc.sync.dma_start_transpose(
                    ht, hcf.rearrange("p c w -> p (c w)"))
                hts[b] = ht

            def p2c(b):
                ht = hts.pop(b)
                osb = work.tile([128, T2, 128], f32, tag="osb", name="osb")
                nc.vector.tensor_tensor(
                    osb.rearrange("p t d -> p (t d)"),
                    x_bf[:, b].rearrange("p t d -> p (t d)"),
                    ht.rearrange("p t d -> p (t d)"), Alu.add)
                nc.sync.dma_start(
                    outd[b].rearrange("(t p) d -> p t d", p=128), osb)

            for i in range(nb + 3):
                if i < nb:
                    p2a(i)
                if 3 <= i <= nb + 2:
                    p2c(i - 3)
                if 1 <= i <= nb:
                    p2b(i - 1)

    nc.compile()
    return nc


def _get_program():
    global _PROGRAM
    if _PROGRAM is None:
        _PROGRAM = _build_program(NB)
    return _PROGRAM


def _prepare_params(inputs):
    bf = ml_dtypes.bfloat16
    g = {k: np.asarray(v, np.float32) for k, v in inputs.items()
         if k not in ("f2", "f3", "f4")}
    pe2, pe3, pe4 = g["pe2"][0], g["pe3"][0], g["pe4"][0]

    def fold_w(ln_w, w):
        return np.ascontiguousarray(ln_w[:, None] * w).astype(bf)

    def fold_bt(ln_b, pe, w, b):
        return np.ascontiguousarray(((ln_b[None, :] + pe) @ w + b[None, :]).T).astype(bf)

    p = {}
    p["wq1"] = fold_w(g["ln1_w"], g["q1_w"])
    p["wq2"] = fold_w(g["ln1_w"], g["q2_w"])
    p["wkv1"] = np.ascontiguousarray(np.concatenate(
        [g["ln2_w"][:, None] * g["k1_w"], g["ln2_w"][:, None] * g["v1_w"]],
        axis=1)).astype(bf)
    p["wkv2"] = np.ascontiguousarray(np.concatenate(
        [g["ln3_w"][:, None] * g["k2_w"], g["ln3_w"][:, None] * g["v2_w"]],
        axis=1)).astype(bf)
    p["bq1t"] = fold_bt(g["ln1_b"], pe2, g["q1_w"], g["q1_b"])
    p["bq2t"] = fold_bt(g["ln1_b"], pe2, g["q2_w"], g["q2_b"])
    bk3 = (g["ln2_b"][None, :] + pe3) @ g["k1_w"] + g["k1_b"][None, :]  # [256,128]
    bv3row = g["ln2_b"] @ g["v1_w"] + g["v1_b"]
    bkv3 = np.zeros((T3, 128, 2 * D), np.float32)
    for t in range(T3):
        bkv3[t, :, :D] = bk3[t * 128 : (t + 1) * 128]
        bkv3[t, :, D:] = bv3row[None, :]
    p["bkv3"] = np.ascontiguousarray(bkv3).astype(bf)
    bk4 = (g["ln3_b"][None, :] + pe4) @ g["k2_w"] + g["k2_b"][None, :]  # [64,128]
    bv4row = g["ln3_b"] @ g["v2_w"] + g["v2_b"]
    bkv4 = np.zeros((128, 2 * D), np.float32)
    bkv4[:64, :D] = bk4
    bkv4[64:, :D] = bk4
    bkv4[:, D:] = bv4row[None, :]
    p["bkv4"] = np.ascontiguousarray(bkv4).astype(bf)
    p["wrp"] = np.ascontiguousarray(g["rp_w"].reshape(2, D, D)).astype(bf)
    p["rpbr"] = np.ascontiguousarray(
        np.tile(g["rp_b"][None, None, :], (1, 4, 1))).astype(bf)
    p["ff2br"] = np.ascontiguousarray(
        np.tile(g["ff2_b"][None, None, :], (1, 4, 1))).astype(bf)
    p["wff1"] = fold_w(g["ln4_w"], g["ff1_w"])
    bff1 = g["ln4_b"] @ g["ff1_w"] + g["ff1_b"]
    p["ff1b"] = np.ascontiguousarray(bff1.reshape(4, 128).T).astype(np.float32)
    p["wff2"] = np.ascontiguousarray(g["ff2_w"].reshape(4, 128, D)).astype(bf)
    p["onesr"] = np.ones((1, 128), np.float32).astype(bf)
    p["ident"] = np.eye(128, dtype=np.float32).astype(bf)
    return p


def kernel(**inputs):
    global LAST_RESULTS
    from concourse import bass_utils

    bf = ml_dtypes.bfloat16
    f2 = np.ascontiguousarray(np.asarray(inputs["f2"]).astype(bf))
    f3 = np.ascontiguousarray(np.asarray(inputs["f3"]).astype(bf))
    f4 = np.ascontiguousarray(np.asarray(inputs["f4"]).astype(bf))
    params = _prepare_params(inputs)
    nc = _get_program()

    in_maps = []
    for c in range(NCORES):
        m = dict(params)
        sl = slice(c * NB, (c + 1) * NB)
        m["f2"] = f2[sl]
        m["f3"] = f3[sl]
        m["f4"] = f4[sl]
        in_maps.append(m)

    res = bass_utils.run_bass_kernel_spmd(
        nc, in_maps, list(range(NCORES)),
        trace=bool(int(os.environ.get("KERNEL_TRACE", "0"))),
    )
    LAST_RESULTS = res
    out = np.concatenate([r["out"] for r in res.results], axis=0)
    return np.ascontiguousarray(out.astype(np.float32))


# revision 29
# speedup vs baseline: 1.1334x; 1.1334x over previous
"""Trainium2 Bass kernel for the CPA block (sparse/efficient attention).

Strategy
--------
Data parallel over batch: B=128 -> 16 batch elements per NeuronCore, all
parameters replicated (folded on host into a handful of small matrices).
Inputs are converted to bf16 on the host (rel-err budget 2e-2 leaves room).

Pass 1 runs a 4-stage software pipeline (front / back1a / back1b / back2,
skewed by one batch each) so no engine ever blocks on a result produced
earlier in the same batch:
  front(b):  load f2/f3 tokens-first into one tile; per-tile bn_stats +
             hand-rolled even/odd aggregation (GpSimd smalls); rstd via
             fast inverse-sqrt (bit trick + 1 Newton step) fully on DVE;
             f4 handled in 2-batch pairs (64 tokens each).
  back1a(b): normalize (tiles split DVE tensor_scalar / ACT Identity with
             scale+bias); z -> zT channels-first via ONE DMA XBAR
             transpose (no PE transposes, no PSUM copies); k3/v3 mms with
             biases via identity matmuls (emitted before q so ACT's
             exp/scale latency hides behind the q matmuls); exp(k);
             per-head sums; 1/s folded into v via ACT Copy+scale;
             q1/q2 = Wq.T @ zT + ident-mm bias; exp with accum_out
             giving softmax denominators for free.
  back1b(b): Gram matmuls; masked block-diag; fold with rp_w on PE; 1/S
             via ACT Copy+scale -> M32/M42; attention TOKENS-FIRST (per
             128-token tile ap += eqT@M32 + eqT@M42 + rank-1 rp_b);
             x_bf = f2 + ap; LN4 stats.
  back2(b):  zx normalize; zxT via DMA transpose.
Pass 2 (Gelu act table) runs a 3-stage pipeline:
  p2a: ff1 -> Gelu ([128,2,512] 2-bank chunks, bias via ACT port)
  p2b: ff2 channels-first (8 wide mms + single-group rank-1 ff2_b),
       bf16 copy, DMA-transpose back to tokens-first
  p2c: out = x_bf + hT -> DMA.

All matmuls bf16 (fp32 PSUM accumulate); residual stream bf16 (rel-err
budget 2e-2; measured ~1.8e-3).
"""

import os

import ml_dtypes
import numpy as np

NB = 16  # batch elements per core
NCORES = 8
EPS = 1e-5
N2, N3, N4, D, MLP = 1024, 256, 64, 128, 512
T2, T3 = N2 // 128, N3 // 128

_PROGRAM = None
LAST_RESULTS = None


def _build_program(nb=NB):
    from contextlib import ExitStack

    import concourse.bacc as bacc
    import concourse.mybir as mybir
    import concourse.tile as tile

    f32 = mybir.dt.float32
    bf16 = mybir.dt.bfloat16
    i32 = mybir.dt.int32
    A = mybir.ActivationFunctionType
    Alu = mybir.AluOpType
    X = mybir.AxisListType.X

    class _Bacc(bacc.Bacc):
        _ACT_SETS = {"natural_log_exp_and_others", "gelu_and_others"}

        def insert_act_table_loads(self):
            import bass_rust as _bass_rust

            from concourse.hw_specs import get_activation_tables

            has_activation = any(
                isinstance(i, mybir.InstActivation)
                for b in self.main_func.blocks
                for i in b.instructions
            )
            if not has_activation:
                return
            tables = [
                (name, (fns if name in self._ACT_SETS else set()))
                for name, fns in get_activation_tables(self.m.arch).items()
            ]
            _bass_rust.insert_act_table_loads(self, tables)

    nc = _Bacc("TRN2", target_bir_lowering=False, debug=False)

    def din(name, shape, dt=bf16):
        return nc.dram_tensor(name, shape, dt, kind="ExternalInput").ap()

    f2d = din("f2", [nb, N2, D])
    f3d = din("f3", [nb, N3, D])
    f4d = din("f4", [nb, N4, D])
    wq1d = din("wq1", [D, D])
    wq2d = din("wq2", [D, D])
    wkv1d = din("wkv1", [D, 2 * D])
    wkv2d = din("wkv2", [D, 2 * D])
    wrpd = din("wrp", [2, D, D])
    wff1d = din("wff1", [D, MLP])
    wff2d = din("wff2", [4, D, D])
    bq1td = din("bq1t", [D, N2])
    bq2td = din("bq2t", [D, N2])
    bkv3d = din("bkv3", [T3, 128, 2 * D])
    bkv4d = din("bkv4", [128, 2 * D])
    rpbrd = din("rpbr", [1, 4, D])
    ff2brd = din("ff2br", [1, 4, D])
    onesrd = din("onesr", [1, 128])
    identd = din("ident", [128, 128])
    ff1bd = din("ff1b", [D, 4], f32)
    outd = nc.dram_tensor("out", [nb, N2, D], f32, kind="ExternalOutput").ap()

    LNEPS = 128.0 * EPS
    C128 = 0.5 * float(np.log(128.0))

    with tile.TileContext(nc) as tc, ExitStack() as ctx:
        consts = ctx.enter_context(tc.tile_pool(name="consts", bufs=1))
        state = ctx.enter_context(tc.tile_pool(name="state", bufs=1))
        work = ctx.enter_context(tc.tile_pool(name="work", bufs=2))
        zpool = ctx.enter_context(tc.tile_pool(name="zpool", bufs=4))
        ftpool = ctx.enter_context(tc.tile_pool(name="ftpool", bufs=3))
        pwork = ctx.enter_context(tc.tile_pool(name="pwork", bufs=2))
        small = ctx.enter_context(tc.tile_pool(name="small", bufs=2))

        def cload(name, shape, dt, src):
            t = consts.tile(shape, dt, name=name)
            nc.sync.dma_start(t, src)
            return t

        wq1 = cload("wq1_sb", [D, D], bf16, wq1d)
        wq2 = cload("wq2_sb", [D, D], bf16, wq2d)
        wkv1 = cload("wkv1_sb", [D, 2 * D], bf16, wkv1d)
        wkv2 = cload("wkv2_sb", [D, 2 * D], bf16, wkv2d)
        wrp = consts.tile([D, 2, D], bf16, name="wrp_sb")
        nc.sync.dma_start(wrp, wrpd.rearrange("g d e -> d g e"))
        wff1 = cload("wff1_sb", [D, MLP], bf16, wff1d)
        wff2 = consts.tile([128, 4, 128], bf16, name="wff2_sb")
        nc.sync.dma_start(wff2, wff2d.rearrange("j k m -> k j m"))
        bq1t = cload("bq1t_sb", [D, N2], bf16, bq1td)
        bq2t = cload("bq2t_sb", [D, N2], bf16, bq2td)
        bkv3 = consts.tile([128, T3, 2 * D], bf16, name="bkv3_sb")
        nc.sync.dma_start(bkv3, bkv3d.rearrange("t p d -> p t d"))
        bkv4 = cload("bkv4_sb", [128, 2 * D], bf16, bkv4d)
        rpbr4 = cload("rpbr_sb", [1, 4, D], bf16, rpbrd)
        ff2br4 = cload("ff2br_sb", [1, 4, D], bf16, ff2brd)
        onesr = cload("onesr_sb", [1, 128], bf16, onesrd)
        ident = cload("ident_sb", [128, 128], bf16, identd)
        ff1b = cload("ff1b_sb", [D, 4], f32, ff1bd)

        magic = consts.tile([128, 16], i32, name="magic")
        nc.vector.memset(magic, 0x5F3759DF)

        x_bf = state.tile([128, nb, T2, 128], bf16, name="x_bf")
        zxT_all = state.tile([128, nb, T2, 128], bf16, name="zxT_all")

        def ln_stats(src_view, G, tag, eng_diy=None):
            """src [128, G, 128] -> (m [128,G] f32, rstd [128,G] f32).

            bn_stats in <=4-tile chunks, even/odd aggregation by hand,
            rstd on ACT with folded constants.
            """
            eng = eng_diy or nc.vector
            st = small.tile([128, G, 6], f32, tag=f"st{tag}", name=f"st{tag}")
            for i in range(G):
                nc.vector.bn_stats(st[:, i, :], src_view[:, i, :])
            sm = small.tile([128, 4, G], f32, tag=f"ag{tag}", name=f"ag{tag}")
            # sm[:,0]=m_e+m_o  sm[:,1]=m_e-m_o  sm[:,2]=32*d^2  sm[:,3]=v128
            eng.tensor_tensor(sm[:, 0, :], st[:, :, 1], st[:, :, 4], Alu.add)
            eng.tensor_tensor(sm[:, 1, :], st[:, :, 1], st[:, :, 4],
                              Alu.subtract)
            eng.tensor_tensor(sm[:, 2, :], sm[:, 1, :], sm[:, 1, :], Alu.mult)
            eng.tensor_scalar(sm[:, 2, :], sm[:, 2, :], 32.0, None, Alu.mult)
            eng.tensor_tensor(sm[:, 3, :], st[:, :, 2], st[:, :, 5], Alu.add)
            v128 = small.tile([128, G], f32, tag=f"v{tag}", name=f"v{tag}")
            eng.tensor_tensor(v128, sm[:, 3, :], sm[:, 2, :], Alu.add)
            m = small.tile([128, G], f32, tag=f"m{tag}", name=f"m{tag}")
            eng.tensor_scalar(m, sm[:, 0, :], 0.5, None, Alu.mult)
            # rstd = 1/sqrt(var + eps) via bit trick + one Newton step, all
            # on DVE (avoids a DVE->ACT->DVE latency round trip per batch).
            wt = small.tile([128, G], f32, tag=f"w{tag}", name=f"w{tag}")
            nc.vector.tensor_scalar(wt, v128, LNEPS, 1.0 / 128.0,
                                    Alu.add, Alu.mult)
            it = small.tile([128, G], i32, tag=f"i{tag}", name=f"i{tag}")
            nc.vector.tensor_scalar(it, wt.bitcast(i32), 1, None,
                                    Alu.logical_shift_right)
            y = small.tile([128, G], f32, tag=f"y{tag}", name=f"y{tag}")
            nc.vector.tensor_tensor(y.bitcast(i32), magic[:, :G], it,
                                    Alu.subtract)
            y2 = small.tile([128, G], f32, tag=f"y2{tag}", name=f"y2{tag}")
            nc.vector.tensor_tensor(y2, y, y, Alu.mult)
            h = small.tile([128, G], f32, tag=f"h{tag}", name=f"h{tag}")
            nc.vector.scalar_tensor_tensor(h, wt, -0.5, y2, Alu.mult, Alu.mult)
            rstd = small.tile([128, G], f32, tag=f"r{tag}", name=f"r{tag}")
            nc.vector.scalar_tensor_tensor(rstd, h, 1.5, y, Alu.add, Alu.mult)
            negmr = small.tile([128, G], f32, tag=f"n{tag}", name=f"n{tag}")
            nc.vector.scalar_tensor_tensor(negmr, m, -1.0, rstd,
                                           Alu.mult, Alu.mult)
            return m, rstd, negmr

        # ---------------- pass 1 (3-stage software pipeline) ----------------
        fronts, mids, pairs, backs = {}, {}, {}, {}
        with tc.tile_pool(name="psq", bufs=2, space="PSUM") as psq, \
             tc.tile_pool(name="pssm", bufs=2, space="PSUM") as pssm, \
             tc.tile_pool(name="psx", bufs=2, space="PSUM") as psx:

            def norm_split(dst, src_v, G, m, r, negmr):
                for t in range(G):
                    if t % 2 == 0:
                        nc.vector.tensor_scalar(dst[:, t, :], src_v[:, t, :],
                                                m[:, t : t + 1],
                                                r[:, t : t + 1],
                                                Alu.subtract, Alu.mult)
                    else:
                        nc.scalar.activation(dst[:, t, :], src_v[:, t, :],
                                             A.Identity,
                                             scale=r[:, t : t + 1],
                                             bias=negmr[:, t : t + 1])

            def front(b):
                """Load + LN stats (results consumed one stage later)."""
                ft = ftpool.tile([128, T2 + T3, 128], bf16, tag="ft", name="ft")
                nc.sync.dma_start(ft[:, :T2, :],
                                  f2d[b].rearrange("(t p) d -> p t d", p=128))
                nc.sync.dma_start(ft[:, T2:, :],
                                  f3d[b].rearrange("(t p) d -> p t d", p=128))
                m23, r23, n23 = ln_stats(ft, T2 + T3, "23")
                fronts[b] = (ft, m23, r23, n23)
                if b % 2 == 0:
                    f4p = pwork.tile([128, 1, 128], bf16, tag="f4p", name="f4p")
                    nc.sync.dma_start(f4p[:, 0, :],
                                      f4d[b : b + 2].rearrange("b n d -> (b n) d"))
                    m4, r4, _n4 = ln_stats(f4p, 1, "4")
                    pairs[b // 2] = [f4p, m4, r4]

            def back1a(b):
                """Normalize + transpose + kv3/q matmuls + exps."""
                ft, m23, r23, n23 = fronts.pop(b)
                if b % 2 == 0:
                    # f4 pair: normalize + kv4 (stats ready since front(b))
                    f4p, m4, r4 = pairs[b // 2]
                    z4p = zpool.tile([128, 128], bf16, tag="z4p", name="z4p")
                    nc.vector.tensor_scalar(z4p, f4p[:, 0, :], m4[:, 0:1],
                                            r4[:, 0:1], Alu.subtract, Alu.mult)
                    z4T = pwork.tile([128, 128], bf16, tag="z4T", name="z4T")
                    nc.sync.dma_start_transpose(z4T, z4p)
                    kv4p = pssm.tile([128, 2 * D], f32, tag="sm", name="kv4p")
                    nc.tensor.matmul(kv4p, z4T, wkv2, start=True, stop=False)
                    nc.tensor.matmul(kv4p, ident, bkv4, start=False, stop=True)
                    ek4 = pwork.tile([128, 128], bf16, tag="ek4", name="ek4")
                    nc.scalar.activation(ek4, kv4p[:, 0:D], A.Exp)
                    s4 = pwork.tile([128, 1, 2], f32, tag="s4", name="s4")
                    nc.vector.tensor_reduce(
                        s4, ek4.rearrange("p (o h e) -> p o h e", o=1, h=2),
                        axis=X, op=Alu.add)
                    nc.vector.reciprocal(s4, s4)
                    v4s = pwork.tile([128, 128], bf16, tag="v4s", name="v4s")
                    for h in range(2):
                        nc.scalar.activation(
                            v4s[:, h * 64 : (h + 1) * 64],
                            kv4p[:, D + h * 64 : D + (h + 1) * 64],
                            A.Copy, scale=s4[:, 0, h : h + 1])
                    pairs[b // 2] = {"ek4": ek4, "v4s": v4s}

                z = zpool.tile([128, T2 + T3, 128], bf16, tag="z", name="z")
                norm_split(z, ft, T2 + T3, m23, r23, n23)
                zT = work.tile([128, T2 + T3, 128], bf16, tag="zT", name="zT")
                nc.sync.dma_start_transpose(
                    zT, z.rearrange("p t d -> p (t d)"))

                # k3/v3 first (so ACT's exp/scale latency hides behind q-mms)
                kv3p = pssm.tile([128, T3, 2 * D], f32, tag="sm", name="kv3p")
                for t in range(T3):
                    nc.tensor.matmul(kv3p[:, t, :], zT[:, T2 + t, :], wkv1,
                                     start=True, stop=False)
                    nc.tensor.matmul(kv3p[:, t, :], ident, bkv3[:, t, :],
                                     start=False, stop=True)
                ek3 = work.tile([128, T3, 128], bf16, tag="ek3", name="ek3")
                nc.scalar.activation(ek3, kv3p[:, :, 0:D], A.Exp)
                s3 = small.tile([128, T3, 2], f32, tag="s3", name="s3")
                nc.vector.tensor_reduce(
                    s3, ek3.rearrange("p t (h e) -> p t h e", h=2),
                    axis=X, op=Alu.add)
                nc.vector.reciprocal(s3, s3)
                v3s = work.tile([128, T3, 128], bf16, tag="v3s", name="v3s")
                for t in range(T3):
                    for h in range(2):
                        nc.scalar.activation(
                            v3s[:, t, h * 64 : (h + 1) * 64],
                            kv3p[:, t, D + h * 64 : D + (h + 1) * 64],
                            A.Copy, scale=s3[:, t, h : h + 1])

                # q projections + exp with accumulated denominators
                S = small.tile([128, 2], f32, tag="S", name="S")
                eqs = []
                for qi, (wq, bqt) in enumerate(((wq1, bq1t), (wq2, bq2t))):
                    qp = psq.tile([128, 2, 512], f32, tag="q", name="qp")
                    for c in range(2):
                        nc.tensor.matmul(qp[:, c, :], wq,
                                         zT[:, c * 4 : (c + 1) * 4, :],
                                         start=True, stop=False)
                        nc.tensor.matmul(qp[:, c, :], ident,
                                         bqt[:, c * 512 : (c + 1) * 512],
                                         start=False, stop=True)
                    eq = work.tile([128, 2, 512], bf16, tag=f"eq{qi}",
                                   name=f"eq{qi}")
                    nc.scalar.activation(eq, qp, A.Exp,
                                         accum_out=S[:, qi : qi + 1])
                    eqs.append(eq)
                rS = small.tile([128, 2], f32, tag="rS", name="rS")
                nc.vector.reciprocal(rS, S)
                mids[b] = (ft, eqs[0], eqs[1], ek3, v3s, rS)

            def back1b(b):
                ft, eq1, eq2, ek3, v3s, rS = mids.pop(b)
                pair = pairs[b // 2]
                # Gram matrices (transposed): g[e,d] = sum_tok v[tok,e] k[tok,d]
                gp = pssm.tile([128, 2, 128], f32, tag="sm", name="gp")
                for t in range(T3):
                    nc.tensor.matmul(gp[:, 0, :], v3s[:, t, :], ek3[:, t, :],
                                     start=(t == 0), stop=(t == T3 - 1))
                off = (b % 2) * 64
                nc.tensor.matmul(gp[:, 1, :],
                                 pair["v4s"][off : off + 64, :],
                                 pair["ek4"][off : off + 64, :],
                                 start=True, stop=True)
                # mask to block-diagonal (GpSimd), fold rp_w and 1/S on PE/DVE
                gmb = work.tile([128, 2, 128], bf16, tag="gmb", name="gmb")
                nc.gpsimd.memset(gmb, 0)
                for h in range(2):
                    nc.scalar.activation(
                        gmb[h * 64 : (h + 1) * 64, :, h * 64 : (h + 1) * 64],
                        gp[h * 64 : (h + 1) * 64, :, h * 64 : (h + 1) * 64],
                        A.Copy)
                mp = pssm.tile([128, 2, 128], f32, tag="sm", name="mp")
                for gi in range(2):
                    nc.tensor.matmul(mp[:, gi, :], gmb[:, gi, :], wrp[:, gi, :],
                                     start=True, stop=True)
                ms = work.tile([128, 2, 128], bf16, tag="ms", name="ms")
                for gi in range(2):
                    nc.scalar.activation(ms[:, gi, :], mp[:, gi, :],
                                         A.Copy, scale=rS[:, gi : gi + 1])

                # attn tokens-first + residual; x_bf = f2 + attn + rp_b
                for half in range(2):
                    ap = psx.tile([128, 4, 128], f32, tag="x", name="ap")
                    for q in range(4):
                        t = half * 4 + q
                        c, w = t // 4, (t % 4) * 128
                        nc.tensor.matmul(ap[:, q, :],
                                         eq1[:, c, w : w + 128], ms[:, 0, :],
                                         start=True, stop=False)
                        nc.tensor.matmul(ap[:, q, :],
                                         eq2[:, c, w : w + 128], ms[:, 1, :],
                                         start=False, stop=False)
                        nc.tensor.matmul(ap[:, q, :], onesr, rpbr4[:, 0, :],
                                         start=False, stop=True)
                    nc.vector.tensor_tensor(
                        x_bf[:, b, half * 4 : (half + 1) * 4, :]
                            .rearrange("p t d -> p (t d)"),
                        ft[:, half * 4 : (half + 1) * 4, :]
                            .rearrange("p t d -> p (t d)"),
                        ap.rearrange("p t d -> p (t d)"), Alu.add)

                # LN4 stats on x_bf (normalize happens one stage later)
                mx, rx, nx = ln_stats(x_bf[:, b], T2, "x")
                backs[b] = (mx, rx, nx)

            def back2(b):
                mx, rx, nx = backs.pop(b)
                zx = zpool.tile([128, T2, 128], bf16, tag="zx", name="zx")
                norm_split(zx, x_bf[:, b], T2, mx, rx, nx)
                nc.sync.dma_start_transpose(
                    zxT_all[:, b], zx.rearrange("p t d -> p (t d)"))

            for i in range(nb + 3):
                if i < nb:
                    front(i)
                if 3 <= i <= nb + 2:
                    back2(i - 3)
                if 2 <= i <= nb + 1:
                    back1b(i - 2)
                if 1 <= i <= nb:
                    back1a(i - 1)

        # ---------------- pass 2 (3-stage pipeline) ----------------
        gjss, hts = {}, {}
        with tc.tile_pool(name="psg", bufs=2, space="PSUM") as psg, \
             tc.tile_pool(name="psh", bufs=2, space="PSUM") as psh:

            def p2a(b):
                zxT = zxT_all[:, b]
                gjs = []
                for j in range(4):
                    gpp = psg.tile([128, 2, 512], f32, tag="g", name="gpp")
                    for c in range(2):
                        nc.tensor.matmul(gpp[:, c, :],
                                         wff1[:, j * 128 : (j + 1) * 128],
                                         zxT[:, c * 4 : (c + 1) * 4, :],
                                         start=True, stop=True)
                    gj = work.tile([128, 2, 512], bf16, tag=f"gj{j}",
                                   name=f"gj{j}")
                    nc.scalar.activation(gj, gpp, A.Gelu,
                                         bias=ff1b[:, j : j + 1])
                    gjs.append(gj)
                gjss[b] = gjs

            def p2b(b):
                # h channels-first: 8 wide matmuls, then bf16 copy + DMA
                # transpose back to tokens-first.
                gjs = gjss.pop(b)
                hq = psh.tile([128, 2, 512], f32, tag="h", name="hq")
                for c in range(2):
                    for j in range(4):
                        nc.tensor.matmul(hq[:, c, :], wff2[:, j, :],
                                         gjs[j][:, c, :],
                                         start=(j == 0), stop=False)
                    nc.tensor.matmul(
                        hq[:, c, :], onesr,
                        ff2br4.rearrange("o f d -> o (f d)"),
                        start=False, stop=True)
                hcf = work.tile([128, 2, 512], bf16, tag="hcf", name="hcf")
                nc.vector.tensor_copy(hcf, hq)
                ht = zpool.tile([128, T2, 128], bf16, tag="ht", name="ht")
                nc.sync.dma_start_transpose(
                    ht, hcf.rearrange("p c w -> p (c w)"))
                hts[b] = ht

            def p2c(b):
                ht = hts.pop(b)
                osb = work.tile([128, T2, 128], f32, tag="osb", name="osb")
                nc.vector.tensor_tensor(
                    osb.rearrange("p t d -> p (t d)"),
                    x_bf[:, b].rearrange("p t d -> p (t d)"),
                    ht.rearrange("p t d -> p (t d)"), Alu.add)
                nc.sync.dma_start(
                    outd[b].rearrange("(t p) d -> p t d", p=128), osb)

            for i in range(nb + 3):
                if i < nb:
                    p2a(i)
                if 3 <= i <= nb + 2:
                    p2c(i - 3)
                if 1 <= i <= nb:
                    p2b(i - 1)

    nc.compile()
    return nc


def _get_program():
    global _PROGRAM
    if _PROGRAM is None:
        _PROGRAM = _build_program(NB)
    return _PROGRAM


def _prepare_params(inputs):
    bf = ml_dtypes.bfloat16
    g = {k: np.asarray(v, np.float32) for k, v in inputs.items()
         if k not in ("f2", "f3", "f4")}
    pe2, pe3, pe4 = g["pe2"][0], g["pe3"][0], g["pe4"][0]

    def fold_w(ln_w, w):
        return np.ascontiguousarray(ln_w[:, None] * w).astype(bf)

    def fold_bt(ln_b, pe, w, b):
        return np.ascontiguousarray(((ln_b[None, :] + pe) @ w + b[None, :]).T).astype(bf)

    p = {}
    p["wq1"] = fold_w(g["ln1_w"], g["q1_w"])
    p["wq2"] = fold_w(g["ln1_w"], g["q2_w"])
    p["wkv1"] = np.ascontiguousarray(np.concatenate(
        [g["ln2_w"][:, None] * g["k1_w"], g["ln2_w"][:, None] * g["v1_w"]],
        axis=1)).astype(bf)
    p["wkv2"] = np.ascontiguousarray(np.concatenate(
        [g["ln3_w"][:, None] * g["k2_w"], g["ln3_w"][:, None] * g["v2_w"]],
        axis=1)).astype(bf)
    p["bq1t"] = fold_bt(g["ln1_b"], pe2, g["q1_w"], g["q1_b"])
    p["bq2t"] = fold_bt(g["ln1_b"], pe2, g["q2_w"], g["q2_b"])
    bk3 = (g["ln2_b"][None, :] + pe3) @ g["k1_w"] + g["k1_b"][None, :]  # [256,128]
    bv3row = g["ln2_b"] @ g["v1_w"] + g["v1_b"]
    bkv3 = np.zeros((T3, 128, 2 * D), np.float32)
    for t in range(T3):
        bkv3[t, :, :D] = bk3[t * 128 : (t + 1) * 128]
        bkv3[t, :, D:] = bv3row[None, :]
    p["bkv3"] = np.ascontiguousarray(bkv3).astype(bf)
    bk4 = (g["ln3_b"][None, :] + pe4) @ g["k2_w"] + g["k2_b"][None, :]  # [64,128]
    bv4row = g["ln3_b"] @ g["v2_w"] + g["v2_b"]
    bkv4 = np.zeros((128, 2 * D), np.float32)
    bkv4[:64, :D] = bk4
    bkv4[64:, :D] = bk4
    bkv4[:, D:] = bv4row[None, :]
    p["bkv4"] = np.ascontiguousarray(bkv4).astype(bf)
    p["wrp"] = np.ascontiguousarray(g["rp_w"].reshape(2, D, D)).astype(bf)
    p["rpbr"] = np.ascontiguousarray(
        np.tile(g["rp_b"][None, None, :], (1, 4, 1))).astype(bf)
    p["ff2br"] = np.ascontiguousarray(
        np.tile(g["ff2_b"][None, None, :], (1, 4, 1))).astype(bf)
    p["wff1"] = fold_w(g["ln4_w"], g["ff1_w"])
    bff1 = g["ln4_b"] @ g["ff1_w"] + g["ff1_b"]
    p["ff1b"] = np.ascontiguousarray(bff1.reshape(4, 128).T).astype(np.float32)
    p["wff2"] = np.ascontiguousarray(g["ff2_w"].reshape(4, 128, D)).astype(bf)
    p["onesr"] = np.ones((1, 128), np.float32).astype(bf)
    p["ident"] = np.eye(128, dtype=np.float32).astype(bf)
    return p


def kernel(**inputs):
    global LAST_RESULTS
    from concourse import bass_utils

    bf = ml_dtypes.bfloat16
    f2 = np.ascontiguousarray(np.asarray(inputs["f2"]).astype(bf))
    f3 = np.ascontiguousarray(np.asarray(inputs["f3"]).astype(bf))
    f4 = np.ascontiguousarray(np.asarray(inputs["f4"]).astype(bf))
    params = _prepare_params(inputs)
    nc = _get_program()

    in_maps = []
    for c in range(NCORES):
        m = dict(params)
        sl = slice(c * NB, (c + 1) * NB)
        m["f2"] = f2[sl]
        m["f3"] = f3[sl]
        m["f4"] = f4[sl]
        in_maps.append(m)

    res = bass_utils.run_bass_kernel_spmd(
        nc, in_maps, list(range(NCORES)),
        trace=bool(int(os.environ.get("KERNEL_TRACE", "0"))),
    )
    LAST_RESULTS = res
    out = np.concatenate([r["out"] for r in res.results], axis=0)
    return np.ascontiguousarray(out.astype(np.float32))


# revision 30
# speedup vs baseline: 1.1676x; 1.0302x over previous
"""Trainium2 Bass kernel for the CPA block (sparse/efficient attention).

Strategy
--------
Data parallel over batch: B=128 -> 16 batch elements per NeuronCore, all
parameters replicated (folded on host into a handful of small matrices).
Inputs are converted to bf16 on the host (rel-err budget 2e-2 leaves room).

Pass 1 runs a 4-stage software pipeline (front / back1a / back1b / back2,
skewed by one batch each) so no engine ever blocks on a result produced
earlier in the same batch:
  front(b):  load f2/f3 tokens-first into one tile; per-tile bn_stats +
             hand-rolled even/odd aggregation (GpSimd smalls); rstd via
             fast inverse-sqrt (bit trick + 1 Newton step) fully on DVE;
             f4 handled in 2-batch pairs (64 tokens each).
  back1a(b): normalize (tiles split DVE tensor_scalar / ACT Identity with
             scale+bias); z -> zT channels-first via ONE DMA XBAR
             transpose (no PE transposes, no PSUM copies); k3/v3 mms with
             biases via identity matmuls (emitted before q so ACT's
             exp/scale latency hides behind the q matmuls); exp(k);
             per-head sums; 1/s folded into v via ACT Copy+scale;
             q1/q2 = Wq.T @ zT + ident-mm bias; exp with accum_out
             giving softmax denominators for free.
  back1b(b): Gram matmuls; masked block-diag; fold with rp_w on PE; 1/S
             via ACT Copy+scale -> M32/M42; attention TOKENS-FIRST (per
             128-token tile ap += eqT@M32 + eqT@M42 + rank-1 rp_b);
             x_bf = f2 + ap; LN4 stats.
  back2(b):  zx normalize; zxT via DMA transpose.
Pass 2 (Gelu act table) runs a 3-stage pipeline:
  p2a: ff1 -> Gelu ([128,2,512] 2-bank chunks, bias via ACT port)
  p2b: ff2 channels-first (8 wide mms + single-group rank-1 ff2_b),
       bf16 copy, DMA-transpose back to tokens-first
  p2c: out = x_bf + hT -> DMA.

All matmuls bf16 (fp32 PSUM accumulate); residual stream bf16 (rel-err
budget 2e-2; measured ~1.8e-3).
"""

import os

import ml_dtypes
import numpy as np

NB = 16  # batch elements per core
NCORES = 8
EPS = 1e-5
N2, N3, N4, D, MLP = 1024, 256, 64, 128, 512
T2, T3 = N2 // 128, N3 // 128

_PROGRAM = None
LAST_RESULTS = None


def _build_program(nb=NB):
    from contextlib import ExitStack

    import concourse.bacc as bacc
    import concourse.mybir as mybir
    import concourse.tile as tile

    f32 = mybir.dt.float32
    bf16 = mybir.dt.bfloat16
    i32 = mybir.dt.int32
    A = mybir.ActivationFunctionType
    Alu = mybir.AluOpType
    X = mybir.AxisListType.X

    class _Bacc(bacc.Bacc):
        _ACT_SETS = {"natural_log_exp_and_others", "gelu_and_others"}

        def insert_act_table_loads(self):
            import bass_rust as _bass_rust

            from concourse.hw_specs import get_activation_tables

            has_activation = any(
                isinstance(i, mybir.InstActivation)
                for b in self.main_func.blocks
                for i in b.instructions
            )
            if not has_activation:
                return
            tables = [
                (name, (fns if name in self._ACT_SETS else set()))
                for name, fns in get_activation_tables(self.m.arch).items()
            ]
            _bass_rust.insert_act_table_loads(self, tables)

    nc = _Bacc("TRN2", target_bir_lowering=False, debug=False)

    def din(name, shape, dt=bf16):
        return nc.dram_tensor(name, shape, dt, kind="ExternalInput").ap()

    f23d = din("f23", [nb, N2 + N3, D])
    f4d = din("f4", [nb, N4, D])
    wq1d = din("wq1", [D, D])
    wq2d = din("wq2", [D, D])
    wkv1d = din("wkv1", [D, 2 * D])
    wkv2d = din("wkv2", [D, 2 * D])
    wrpd = din("wrp", [2, D, D])
    wff1d = din("wff1", [D, MLP])
    wff2d = din("wff2", [4, D, D])
    bq1td = din("bq1t", [D, N2])
    bq2td = din("bq2t", [D, N2])
    bkv3d = din("bkv3", [T3, 128, 2 * D])
    bkv4d = din("bkv4", [128, 2 * D])
    rpbrd = din("rpbr", [1, 4, D])
    ff2brd = din("ff2br", [1, 4, D])
    onesrd = din("onesr", [1, 128])
    identd = din("ident", [128, 128])
    ff1bd = din("ff1b", [D, 4], f32)
    outd = nc.dram_tensor("out", [nb, N2, D], f32, kind="ExternalOutput").ap()

    LNEPS = 128.0 * EPS
    C128 = 0.5 * float(np.log(128.0))

    with tile.TileContext(nc) as tc, ExitStack() as ctx:
        consts = ctx.enter_context(tc.tile_pool(name="consts", bufs=1))
        state = ctx.enter_context(tc.tile_pool(name="state", bufs=1))
        work = ctx.enter_context(tc.tile_pool(name="work", bufs=2))
        zpool = ctx.enter_context(tc.tile_pool(name="zpool", bufs=4))
        ftpool = ctx.enter_context(tc.tile_pool(name="ftpool", bufs=3))
        pwork = ctx.enter_context(tc.tile_pool(name="pwork", bufs=2))
        small = ctx.enter_context(tc.tile_pool(name="small", bufs=2))

        def cload(name, shape, dt, src):
            t = consts.tile(shape, dt, name=name)
            nc.sync.dma_start(t, src)
            return t

        wq1 = cload("wq1_sb", [D, D], bf16, wq1d)
        wq2 = cload("wq2_sb", [D, D], bf16, wq2d)
        wkv1 = cload("wkv1_sb", [D, 2 * D], bf16, wkv1d)
        wkv2 = cload("wkv2_sb", [D, 2 * D], bf16, wkv2d)
        wrp = consts.tile([D, 2, D], bf16, name="wrp_sb")
        nc.sync.dma_start(wrp, wrpd.rearrange("g d e -> d g e"))
        wff1 = cload("wff1_sb", [D, MLP], bf16, wff1d)
        wff2 = consts.tile([128, 4, 128], bf16, name="wff2_sb")
        nc.sync.dma_start(wff2, wff2d.rearrange("j k m -> k j m"))
        bq1t = cload("bq1t_sb", [D, N2], bf16, bq1td)
        bq2t = cload("bq2t_sb", [D, N2], bf16, bq2td)
        bkv3 = consts.tile([128, T3, 2 * D], bf16, name="bkv3_sb")
        nc.sync.dma_start(bkv3, bkv3d.rearrange("t p d -> p t d"))
        bkv4 = cload("bkv4_sb", [128, 2 * D], bf16, bkv4d)
        rpbr4 = cload("rpbr_sb", [1, 4, D], bf16, rpbrd)
        ff2br4 = cload("ff2br_sb", [1, 4, D], bf16, ff2brd)
        onesr = cload("onesr_sb", [1, 128], bf16, onesrd)
        ident = cload("ident_sb", [128, 128], bf16, identd)
        ff1b = cload("ff1b_sb", [D, 4], f32, ff1bd)

        magic = consts.tile([128, 16], i32, name="magic")
        nc.vector.memset(magic, 0x5F3759DF)

        x_bf = state.tile([128, nb, T2, 128], bf16, name="x_bf")
        zxT_all = state.tile([128, nb, T2, 128], bf16, name="zxT_all")

        def ln_stats(src_view, G, tag, eng_diy=None):
            """src [128, G, 128] -> (m [128,G] f32, rstd [128,G] f32).

            bn_stats in <=4-tile chunks, even/odd aggregation by hand,
            rstd on ACT with folded constants.
            """
            eng = eng_diy or nc.vector
            st = small.tile([128, G, 6], f32, tag=f"st{tag}", name=f"st{tag}")
            for i in range(G):
                nc.vector.bn_stats(st[:, i, :], src_view[:, i, :])
            sm = small.tile([128, 4, G], f32, tag=f"ag{tag}", name=f"ag{tag}")
            # sm[:,0]=m_e+m_o  sm[:,1]=m_e-m_o  sm[:,2]=32*d^2  sm[:,3]=v128
            eng.tensor_tensor(sm[:, 0, :], st[:, :, 1], st[:, :, 4], Alu.add)
            eng.tensor_tensor(sm[:, 1, :], st[:, :, 1], st[:, :, 4],
                              Alu.subtract)
            eng.tensor_tensor(sm[:, 2, :], sm[:, 1, :], sm[:, 1, :], Alu.mult)
            eng.tensor_scalar(sm[:, 2, :], sm[:, 2, :], 32.0, None, Alu.mult)
            eng.tensor_tensor(sm[:, 3, :], st[:, :, 2], st[:, :, 5], Alu.add)
            v128 = small.tile([128, G], f32, tag=f"v{tag}", name=f"v{tag}")
            eng.tensor_tensor(v128, sm[:, 3, :], sm[:, 2, :], Alu.add)
            m = small.tile([128, G], f32, tag=f"m{tag}", name=f"m{tag}")
            eng.tensor_scalar(m, sm[:, 0, :], 0.5, None, Alu.mult)
            # rstd = 1/sqrt(var + eps) via bit trick + one Newton step, all
            # on DVE (avoids a DVE->ACT->DVE latency round trip per batch).
            wt = small.tile([128, G], f32, tag=f"w{tag}", name=f"w{tag}")
            nc.vector.tensor_scalar(wt, v128, LNEPS, 1.0 / 128.0,
                                    Alu.add, Alu.mult)
            it = small.tile([128, G], i32, tag=f"i{tag}", name=f"i{tag}")
            nc.vector.tensor_scalar(it, wt.bitcast(i32), 1, None,
                                    Alu.logical_shift_right)
            y = small.tile([128, G], f32, tag=f"y{tag}", name=f"y{tag}")
            nc.vector.tensor_tensor(y.bitcast(i32), magic[:, :G], it,
                                    Alu.subtract)
            y2 = small.tile([128, G], f32, tag=f"y2{tag}", name=f"y2{tag}")
            nc.vector.tensor_tensor(y2, y, y, Alu.mult)
            h = small.tile([128, G], f32, tag=f"h{tag}", name=f"h{tag}")
            nc.vector.scalar_tensor_tensor(h, wt, -0.5, y2, Alu.mult, Alu.mult)
            rstd = small.tile([128, G], f32, tag=f"r{tag}", name=f"r{tag}")
            nc.vector.scalar_tensor_tensor(rstd, h, 1.5, y, Alu.add, Alu.mult)
            negmr = small.tile([128, G], f32, tag=f"n{tag}", name=f"n{tag}")
            nc.vector.scalar_tensor_tensor(negmr, m, -1.0, rstd,
                                           Alu.mult, Alu.mult)
            return m, rstd, negmr

        # ---------------- pass 1 (3-stage software pipeline) ----------------
        fronts, mids, pairs, backs = {}, {}, {}, {}
        with tc.tile_pool(name="psq", bufs=2, space="PSUM") as psq, \
             tc.tile_pool(name="pssm", bufs=2, space="PSUM") as pssm, \
             tc.tile_pool(name="psx", bufs=2, space="PSUM") as psx:

            def norm_split(dst, src_v, G, m, r, negmr):
                for t in range(G):
                    if t % 2 == 0:
                        nc.vector.tensor_scalar(dst[:, t, :], src_v[:, t, :],
                                                m[:, t : t + 1],
                                                r[:, t : t + 1],
                                                Alu.subtract, Alu.mult)
                    else:
                        nc.scalar.activation(dst[:, t, :], src_v[:, t, :],
                                             A.Identity,
                                             scale=r[:, t : t + 1],
                                             bias=negmr[:, t : t + 1])

            def front(b):
                """Load + LN stats (results consumed one stage later)."""
                ft = ftpool.tile([128, T2 + T3, 128], bf16, tag="ft", name="ft")
                nc.sync.dma_start(
                    ft, f23d[b].rearrange("(t p) d -> p t d", p=128))
                m23, r23, n23 = ln_stats(ft, T2 + T3, "23")
                fronts[b] = (ft, m23, r23, n23)
                if b % 2 == 0:
                    f4p = pwork.tile([128, 1, 128], bf16, tag="f4p", name="f4p")
                    nc.sync.dma_start(f4p[:, 0, :],
                                      f4d[b : b + 2].rearrange("b n d -> (b n) d"))
                    m4, r4, _n4 = ln_stats(f4p, 1, "4")
                    pairs[b // 2] = [f4p, m4, r4]

            def back1a(b):
                """Normalize + transpose + kv3/q matmuls + exps."""
                ft, m23, r23, n23 = fronts.pop(b)
                if b % 2 == 0:
                    # f4 pair: normalize + kv4 (stats ready since front(b))
                    f4p, m4, r4 = pairs[b // 2]
                    z4p = zpool.tile([128, 128], bf16, tag="z4p", name="z4p")
                    nc.vector.tensor_scalar(z4p, f4p[:, 0, :], m4[:, 0:1],
                                            r4[:, 0:1], Alu.subtract, Alu.mult)
                    z4T = pwork.tile([128, 128], bf16, tag="z4T", name="z4T")
                    nc.sync.dma_start_transpose(z4T, z4p)
                    kv4p = pssm.tile([128, 2 * D], f32, tag="sm", name="kv4p")
                    nc.tensor.matmul(kv4p, z4T, wkv2, start=True, stop=False)
                    nc.tensor.matmul(kv4p, ident, bkv4, start=False, stop=True)
                    ek4 = pwork.tile([128, 128], bf16, tag="ek4", name="ek4")
                    nc.scalar.activation(ek4, kv4p[:, 0:D], A.Exp)
                    s4 = pwork.tile([128, 1, 2], f32, tag="s4", name="s4")
                    nc.vector.tensor_reduce(
                        s4, ek4.rearrange("p (o h e) -> p o h e", o=1, h=2),
                        axis=X, op=Alu.add)
                    nc.vector.reciprocal(s4, s4)
                    v4s = pwork.tile([128, 128], bf16, tag="v4s", name="v4s")
                    for h in range(2):
                        nc.scalar.activation(
                            v4s[:, h * 64 : (h + 1) * 64],
                            kv4p[:, D + h * 64 : D + (h + 1) * 64],
                            A.Copy, scale=s4[:, 0, h : h + 1])
                    pairs[b // 2] = {"ek4": ek4, "v4s": v4s}

                z = zpool.tile([128, T2 + T3, 128], bf16, tag="z", name="z")
                norm_split(z, ft, T2 + T3, m23, r23, n23)
                zT = work.tile([128, T2 + T3, 128], bf16, tag="zT", name="zT")
                nc.sync.dma_start_transpose(
                    zT, z.rearrange("p t d -> p (t d)"))

                # k3/v3 first (so ACT's exp/scale latency hides behind q-mms)
                kv3p = pssm.tile([128, T3, 2 * D], f32, tag="sm", name="kv3p")
                for t in range(T3):
                    nc.tensor.matmul(kv3p[:, t, :], zT[:, T2 + t, :], wkv1,
                                     start=True, stop=False)
                    nc.tensor.matmul(kv3p[:, t, :], ident, bkv3[:, t, :],
                                     start=False, stop=True)
                ek3 = work.tile([128, T3, 128], bf16, tag="ek3", name="ek3")
                nc.scalar.activation(ek3, kv3p[:, :, 0:D], A.Exp)
                s3 = small.tile([128, T3, 2], f32, tag="s3", name="s3")
                nc.vector.tensor_reduce(
                    s3, ek3.rearrange("p t (h e) -> p t h e", h=2),
                    axis=X, op=Alu.add)
                nc.vector.reciprocal(s3, s3)
                v3s = work.tile([128, T3, 128], bf16, tag="v3s", name="v3s")
                for t in range(T3):
                    for h in range(2):
                        nc.scalar.activation(
                            v3s[:, t, h * 64 : (h + 1) * 64],
                            kv3p[:, t, D + h * 64 : D + (h + 1) * 64],
                            A.Copy, scale=s3[:, t, h : h + 1])

                # q projections + exp with accumulated denominators
                S = small.tile([128, 2], f32, tag="S", name="S")
                eqs = []
                for qi, (wq, bqt) in enumerate(((wq1, bq1t), (wq2, bq2t))):
                    qp = psq.tile([128, 2, 512], f32, tag="q", name="qp")
                    for c in range(2):
                        nc.tensor.matmul(qp[:, c, :], wq,
                                         zT[:, c * 4 : (c + 1) * 4, :],
                                         start=True, stop=False)
                        nc.tensor.matmul(qp[:, c, :], ident,
                                         bqt[:, c * 512 : (c + 1) * 512],
                                         start=False, stop=True)
                    eq = work.tile([128, 2, 512], bf16, tag=f"eq{qi}",
                                   name=f"eq{qi}")
                    nc.scalar.activation(eq, qp, A.Exp,
                                         accum_out=S[:, qi : qi + 1])
                    eqs.append(eq)
                rS = small.tile([128, 2], f32, tag="rS", name="rS")
                nc.vector.reciprocal(rS, S)
                mids[b] = (ft, eqs[0], eqs[1], ek3, v3s, rS)

            def back1b(b):
                ft, eq1, eq2, ek3, v3s, rS = mids.pop(b)
                pair = pairs[b // 2]
                # Gram matrices (transposed): g[e,d] = sum_tok v[tok,e] k[tok,d]
                gp = pssm.tile([128, 2, 128], f32, tag="sm", name="gp")
                for t in range(T3):
                    nc.tensor.matmul(gp[:, 0, :], v3s[:, t, :], ek3[:, t, :],
                                     start=(t == 0), stop=(t == T3 - 1))
                off = (b % 2) * 64
                nc.tensor.matmul(gp[:, 1, :],
                                 pair["v4s"][off : off + 64, :],
                                 pair["ek4"][off : off + 64, :],
                                 start=True, stop=True)
                # mask to block-diagonal (GpSimd), fold rp_w and 1/S on PE/DVE
                gmb = work.tile([128, 2, 128], bf16, tag="gmb", name="gmb")
                nc.gpsimd.memset(gmb, 0)
                for h in range(2):
                    nc.scalar.activation(
                        gmb[h * 64 : (h + 1) * 64, :, h * 64 : (h + 1) * 64],
                        gp[h * 64 : (h + 1) * 64, :, h * 64 : (h + 1) * 64],
                        A.Copy)
                mp = pssm.tile([128, 2, 128], f32, tag="sm", name="mp")
                for gi in range(2):
                    nc.tensor.matmul(mp[:, gi, :], gmb[:, gi, :], wrp[:, gi, :],
                                     start=True, stop=True)
                ms = work.tile([128, 2, 128], bf16, tag="ms", name="ms")
                for gi in range(2):
                    nc.scalar.activation(ms[:, gi, :], mp[:, gi, :],
                                         A.Copy, scale=rS[:, gi : gi + 1])

                # attn tokens-first + residual; x_bf = f2 + attn + rp_b
                for half in range(2):
                    ap = psx.tile([128, 4, 128], f32, tag="x", name="ap")
                    for q in range(4):
                        t = half * 4 + q
                        c, w = t // 4, (t % 4) * 128
                        nc.tensor.matmul(ap[:, q, :],
                                         eq1[:, c, w : w + 128], ms[:, 0, :],
                                         start=True, stop=False)
                        nc.tensor.matmul(ap[:, q, :],
                                         eq2[:, c, w : w + 128], ms[:, 1, :],
                                         start=False, stop=False)
                        nc.tensor.matmul(ap[:, q, :], onesr, rpbr4[:, 0, :],
                                         start=False, stop=True)
                    nc.vector.tensor_tensor(
                        x_bf[:, b, half * 4 : (half + 1) * 4, :]
                            .rearrange("p t d -> p (t d)"),
                        ft[:, half * 4 : (half + 1) * 4, :]
                            .rearrange("p t d -> p (t d)"),
                        ap.rearrange("p t d -> p (t d)"), Alu.add)

                # LN4 stats on x_bf (normalize happens one stage later)
                mx, rx, nx = ln_stats(x_bf[:, b], T2, "x")
                backs[b] = (mx, rx, nx)

            def back2(b):
                mx, rx, nx = backs.pop(b)
                zx = zpool.tile([128, T2, 128], bf16, tag="zx", name="zx")
                norm_split(zx, x_bf[:, b], T2, mx, rx, nx)
                nc.sync.dma_start_transpose(
                    zxT_all[:, b], zx.rearrange("p t d -> p (t d)"))

            for i in range(nb + 3):
                if i < nb:
                    front(i)
                if 3 <= i <= nb + 2:
                    back2(i - 3)
                if 2 <= i <= nb + 1:
                    back1b(i - 2)
                if 1 <= i <= nb:
                    back1a(i - 1)

        # ---------------- pass 2 (3-stage pipeline) ----------------
        gjss, hts = {}, {}
        with tc.tile_pool(name="psg", bufs=2, space="PSUM") as psg, \
             tc.tile_pool(name="psh", bufs=2, space="PSUM") as psh:

            def p2a(b):
                zxT = zxT_all[:, b]
                gjs = []
                for j in range(4):
                    gpp = psg.tile([128, 2, 512], f32, tag="g", name="gpp")
                    for c in range(2):
                        nc.tensor.matmul(gpp[:, c, :],
                                         wff1[:, j * 128 : (j + 1) * 128],
                                         zxT[:, c * 4 : (c + 1) * 4, :],
                                         start=True, stop=True)
                    gj = work.tile([128, 2, 512], bf16, tag=f"gj{j}",
                                   name=f"gj{j}")
                    nc.scalar.activation(gj, gpp, A.Gelu,
                                         bias=ff1b[:, j : j + 1])
                    gjs.append(gj)
                gjss[b] = gjs

            def p2b(b):
                # h channels-first: 8 wide matmuls, then bf16 copy + DMA
                # transpose back to tokens-first.
                gjs = gjss.pop(b)
                hq = psh.tile([128, 2, 512], f32, tag="h", name="hq")
                for c in range(2):
                    for j in range(4):
                        nc.tensor.matmul(hq[:, c, :], wff2[:, j, :],
                                         gjs[j][:, c, :],
                                         start=(j == 0), stop=False)
                    nc.tensor.matmul(
                        hq[:, c, :], onesr,
                        ff2br4.rearrange("o f d -> o (f d)"),
                        start=False, stop=True)
                hcf = work.tile([128, 2, 512], bf16, tag="hcf", name="hcf")
                nc.vector.tensor_copy(hcf, hq)
                ht = zpool.tile([128, T2, 128], bf16, tag="ht", name="ht")
                nc.sync.dma_start_transpose(
                    ht, hcf.rearrange("p c w -> p (c w)"))
                hts[b] = ht

            def p2c(b):
                ht = hts.pop(b)
                osb = work.tile([128, T2, 128], f32, tag="osb", name="osb")
                nc.vector.tensor_tensor(
                    osb.rearrange("p t d -> p (t d)"),
                    x_bf[:, b].rearrange("p t d -> p (t d)"),
                    ht.rearrange("p t d -> p (t d)"), Alu.add)
                nc.sync.dma_start(
                    outd[b].rearrange("(t p) d -> p t d", p=128), osb)

            for i in range(nb + 3):
                if i < nb:
                    p2a(i)
                if 3 <= i <= nb + 2:
                    p2c(i - 3)
                if 1 <= i <= nb:
                    p2b(i - 1)

    nc.compile()
    return nc


def _get_program():
    global _PROGRAM
    if _PROGRAM is None:
        _PROGRAM = _build_program(NB)
    return _PROGRAM


def _prepare_params(inputs):
    bf = ml_dtypes.bfloat16
    g = {k: np.asarray(v, np.float32) for k, v in inputs.items()
         if k not in ("f2", "f3", "f4")}
    pe2, pe3, pe4 = g["pe2"][0], g["pe3"][0], g["pe4"][0]

    def fold_w(ln_w, w):
        return np.ascontiguousarray(ln_w[:, None] * w).astype(bf)

    def fold_bt(ln_b, pe, w, b):
        return np.ascontiguousarray(((ln_b[None, :] + pe) @ w + b[None, :]).T).astype(bf)

    p = {}
    p["wq1"] = fold_w(g["ln1_w"], g["q1_w"])
    p["wq2"] = fold_w(g["ln1_w"], g["q2_w"])
    p["wkv1"] = np.ascontiguousarray(np.concatenate(
        [g["ln2_w"][:, None] * g["k1_w"], g["ln2_w"][:, None] * g["v1_w"]],
        axis=1)).astype(bf)
    p["wkv2"] = np.ascontiguousarray(np.concatenate(
        [g["ln3_w"][:, None] * g["k2_w"], g["ln3_w"][:, None] * g["v2_w"]],
        axis=1)).astype(bf)
    p["bq1t"] = fold_bt(g["ln1_b"], pe2, g["q1_w"], g["q1_b"])
    p["bq2t"] = fold_bt(g["ln1_b"], pe2, g["q2_w"], g["q2_b"])
    bk3 = (g["ln2_b"][None, :] + pe3) @ g["k1_w"] + g["k1_b"][None, :]  # [256,128]
    bv3row = g["ln2_b"] @ g["v1_w"] + g["v1_b"]
    bkv3 = np.zeros((T3, 128, 2 * D), np.float32)
    for t in range(T3):
        bkv3[t, :, :D] = bk3[t * 128 : (t + 1) * 128]
        bkv3[t, :, D:] = bv3row[None, :]
    p["bkv3"] = np.ascontiguousarray(bkv3).astype(bf)
    bk4 = (g["ln3_b"][None, :] + pe4) @ g["k2_w"] + g["k2_b"][None, :]  # [64,128]
    bv4row = g["ln3_b"] @ g["v2_w"] + g["v2_b"]
    bkv4 = np.zeros((128, 2 * D), np.float32)
    bkv4[:64, :D] = bk4
    bkv4[64:, :D] = bk4
    bkv4[:, D:] = bv4row[None, :]
    p["bkv4"] = np.ascontiguousarray(bkv4).astype(bf)
    p["wrp"] = np.ascontiguousarray(g["rp_w"].reshape(2, D, D)).astype(bf)
    p["rpbr"] = np.ascontiguousarray(
        np.tile(g["rp_b"][None, None, :], (1, 4, 1))).astype(bf)
    p["ff2br"] = np.ascontiguousarray(
        np.tile(g["ff2_b"][None, None, :], (1, 4, 1))).astype(bf)
    p["wff1"] = fold_w(g["ln4_w"], g["ff1_w"])
    bff1 = g["ln4_b"] @ g["ff1_w"] + g["ff1_b"]
    p["ff1b"] = np.ascontiguousarray(bff1.reshape(4, 128).T).astype(np.float32)
    p["wff2"] = np.ascontiguousarray(g["ff2_w"].reshape(4, 128, D)).astype(bf)
    p["onesr"] = np.ones((1, 128), np.float32).astype(bf)
    p["ident"] = np.eye(128, dtype=np.float32).astype(bf)
    return p


def kernel(**inputs):
    global LAST_RESULTS
    from concourse import bass_utils

    bf = ml_dtypes.bfloat16
    f23 = np.ascontiguousarray(np.concatenate(
        [np.asarray(inputs["f2"]), np.asarray(inputs["f3"])],
        axis=1).astype(bf))
    f4 = np.ascontiguousarray(np.asarray(inputs["f4"]).astype(bf))
    params = _prepare_params(inputs)
    nc = _get_program()

    in_maps = []
    for c in range(NCORES):
        m = dict(params)
        sl = slice(c * NB, (c + 1) * NB)
        m["f23"] = f23[sl]
        m["f4"] = f4[sl]
        in_maps.append(m)

    res = bass_utils.run_bass_kernel_spmd(
        nc, in_maps, list(range(NCORES)),
        trace=bool(int(os.environ.get("KERNEL_TRACE", "0"))),
    )
    LAST_RESULTS = res
    out = np.concatenate([r["out"] for r in res.results], axis=0)
    return np.ascontiguousarray(out.astype(np.float32))
